# revision 3
# baseline (speedup 1.0000x reference)
"""BinaryConv2d (3x3, pad=1 with PAD_VALUE=-1, stride 1) on 8 TRN2 NeuronCores.

Strategy: data-parallel over batch (4 images per core), binarized weight
replicated. Conv is computed as implicit GEMM: for each of the 9 kernel
positions, a [ic=128 x oc=128] stationary matmul tile multiplies a shifted
window of the padded input, accumulating in PSUM over the 9 positions x 2
ic-chunks (K=256).

Host-side prep: pad x with -1 (exact in bf16), cast to bf16 (weights +-1 are
exact in bf16; accumulation is fp32 in PSUM), lay out weight as
[icc, ic, kpos, oc] so each lhsT tile is a contiguous [128, 128] slice.
"""

import numpy as np
import ml_dtypes
from contextlib import ExitStack

import concourse.bass as bass
import concourse.mybir as mybir
import concourse.tile as tile
from concourse import bacc
from concourse.bass_utils import run_bass_kernel_spmd

N_CORES = 8
B, C, H, W = 32, 256, 56, 56
KH, KW = 3, 3
HP, WP = H + 2, W + 2            # 58 (pad=1 each side)
IMGS_PER_CORE = B // N_CORES     # 4
P = 128
ICC = C // P                     # 2 ic chunks
OCC = C // P                     # 2 oc chunks
KPOS = KH * KW                   # 9
ROWS_PER_BLK = 8
N_BLK = H // ROWS_PER_BLK        # 7
N_FREE = ROWS_PER_BLK * W        # 448 <= 512 (one PSUM bank)

BF16 = mybir.dt.bfloat16
F32 = mybir.dt.float32

_NC_CACHE = {}


def build_nc(n_img=IMGS_PER_CORE):
    """Build the per-core Bass program (same program on every core)."""
    if n_img in _NC_CACHE:
        return _NC_CACHE[n_img]

    nc = bacc.Bacc("TRN2", target_bir_lowering=False, debug=False)
    x_d = nc.declare_dram_parameter("x", [n_img, ICC, P, HP, WP], BF16, isOutput=False)
    w_d = nc.declare_dram_parameter("w", [ICC, P, KPOS, OCC * P], BF16, isOutput=False)
    o_d = nc.declare_dram_parameter("out", [n_img, OCC * P, H, W], F32, isOutput=True)

    with tile.TileContext(nc) as tc, ExitStack() as ctx:
        # bufs=1: every tile here has a unique name/tag and stays resident
        wp = ctx.enter_context(tc.tile_pool(name="w", bufs=1))
        xp = ctx.enter_context(tc.tile_pool(name="x", bufs=1))
        op = ctx.enter_context(tc.tile_pool(name="o", bufs=6))
        pp = ctx.enter_context(tc.tile_pool(name="psum", bufs=8, space="PSUM"))

        w_sb = []
        for icc in range(ICC):
            t = wp.tile([P, KPOS, OCC * P], BF16, name=f"w{icc}")
            nc.sync.dma_start(t[:], w_d[icc])
            w_sb.append(t)

        x_sb = []
        for img in range(n_img):
            per_img = []
            for icc in range(ICC):
                t = xp.tile([P, HP, WP], BF16, name=f"x{img}_{icc}")
                nc.sync.dma_start(t[:], x_d[img, icc])
                per_img.append(t)
            x_sb.append(per_img)

        for img in range(n_img):
            for occ in range(OCC):
                psums = [pp.tile([P, ROWS_PER_BLK, W], F32, name=f"ps{rb}", tag="ps")
                         for rb in range(N_BLK)]
                # weight-stationary inner loop: one lhsT serves N_BLK matmuls
                for ki in range(KPOS):
                    kh, kw = divmod(ki, KW)
                    for icc in range(ICC):
                        lhsT = w_sb[icc][:, ki, occ * P:(occ + 1) * P]
                        start = (ki == 0 and icc == 0)
                        stop = (ki == KPOS - 1 and icc == ICC - 1)
                        for rb in range(N_BLK):
                            r0 = rb * ROWS_PER_BLK + kh
                            rhs = x_sb[img][icc][:, r0:r0 + ROWS_PER_BLK, kw:kw + W]
                            nc.tensor.matmul(
                                psums[rb][:], lhsT, rhs, start=start, stop=stop
                            )
                for rb in range(N_BLK):
                    ot = op.tile([P, ROWS_PER_BLK, W], F32, name=f"ot{rb}", tag="ot")
                    nc.vector.tensor_copy(out=ot[:], in_=psums[rb][:])
                    nc.sync.dma_start(
                        o_d[img, occ * P:(occ + 1) * P,
                            rb * ROWS_PER_BLK:(rb + 1) * ROWS_PER_BLK, :],
                        ot[:],
                    )

    nc.compile()
    _NC_CACHE[n_img] = nc
    return nc


def prep_inputs(x, weight):
    """Host-side shard/layout prep. Returns per-core in_maps."""
    bf16 = ml_dtypes.bfloat16
    # binarize weight (sign with sign(0) -> +1), lay out as [icc, ic, kpos, oc]
    wsign = np.where(weight >= 0, np.float32(1.0), np.float32(-1.0))
    wt = (
        wsign.reshape(OCC, P, ICC, P, KH, KW)
        .transpose(2, 3, 4, 5, 0, 1)
        .reshape(ICC, P, KPOS, OCC * P)
        .astype(bf16)
    )
    # pad with -1, cast to bf16
    xp_all = np.full((B, C, HP, WP), -1.0, dtype=np.float32)
    xp_all[:, :, 1:1 + H, 1:1 + W] = x
    xp_all = xp_all.astype(bf16)

    in_maps = []
    for c in range(N_CORES):
        shard = xp_all[c * IMGS_PER_CORE:(c + 1) * IMGS_PER_CORE]
        shard = np.ascontiguousarray(shard).reshape(IMGS_PER_CORE, ICC, P, HP, WP)
        in_maps.append({"x": shard, "w": wt})
    return in_maps


def run(x, weight, trace=False, **kwargs):
    nc = build_nc()
    in_maps = prep_inputs(x, weight)
    res = run_bass_kernel_spmd(
        nc, in_maps, core_ids=list(range(N_CORES)), trace=trace, **kwargs
    )
    out = np.concatenate([r["out"] for r in res.results], axis=0)
    return out, res


def kernel(x, weight):
    out, _ = run(x, weight, trace=False)
    return out
